# revision 6
# baseline (speedup 1.0000x reference)
"""GCN heat-kernel diffusion (10 hops) + Linear on 8 Trainium2 NeuronCores.

Algorithm (matches reference):
    A_hat = D^-1/2 (Adj + I) D^-1/2
    out = (e^-t * sum_k t^k/k! A_hat^k x) @ W.T + b

Device mapping:
  - Nodes sharded across 8 cores (6250 dst rows each, 49 tiles of <=128).
  - State g_k = dinv * h_k kept replicated in DRAM as two bf16 tables
    (node halves, so gather indices fit int16):
        tableA rows: rank-major [8 x 3072], tableB: [8 x 3178].
  - Per hop per core: dma_gather g[src] rows (4 SWDGE queues), segment-sum
    via one-hot matmuls on the TensorEngine (S matrices precomputed on host
    and streamed from DRAM), finalize h = dinv*(segsum), accumulate
    out += c_k*h, write g_next = dinv*h, AllGather the two table halves.
  - Final: out @ W.T + b via PE transpose + matmul.
"""
import sys

sys.path.insert(0, "/opt/trn_rl_repo")

import numpy as np
import ml_dtypes

import concourse.bass as bass
import concourse.bacc as bacc
import concourse.tile as tile
from concourse import mybir
from concourse.bass_utils import run_bass_kernel_spmd
from concourse.masks import make_identity

FP32 = mybir.dt.float32
BF16 = mybir.dt.bfloat16
I16 = mybir.dt.int16
BF = ml_dtypes.bfloat16

N_CORES = 8
N = 50000
D = 128
K_HOPS = 10
RPC = N // N_CORES            # 6250 rows per core
TPC = 49                      # dst tiles per core (48*128 + 106)
HA = 3072                     # rows of each core region in table A (24 tiles)
HB = RPC - HA                 # 3178 rows in table B
TA_ROWS = N_CORES * HA        # 24576
TB_ROWS = N_CORES * HB        # 25424
A_TILES = HA // 128           # 24 tiles fully in half A
CPS = 8                       # chunks per (tile, half) slot: 1024 edge slots
SLOT = CPS * 128
TILE_BATCH = 4                # tiles per gather batch
P = 128

_BATCHES = [list(range(b, min(b + TILE_BATCH, TPC)))
            for b in range(0, TPC, TILE_BATCH)]  # 13 batches: 12x4 + 1x1


def _build_program(cps):
    slot = cps * 128
    nc = bacc.Bacc("TRN2", target_bir_lowering=False, debug=False,
                   num_devices=N_CORES, num_swdge_queues=4)
    nchunk = TPC * 2 * cps
    sw = TILE_BATCH * cps * 128   # widest S strip (4096 cols)
    tA0 = nc.dram_tensor("tA0", [TA_ROWS, D], BF16, kind="ExternalInput").ap()
    tB0 = nc.dram_tensor("tB0", [TB_ROWS, D], BF16, kind="ExternalInput").ap()
    idx_d = nc.dram_tensor("idx", [P, nchunk * 8], I16, kind="ExternalInput").ap()
    dloc_d = nc.dram_tensor("dlocs", [P, nchunk], FP32, kind="ExternalInput").ap()
    iota_d = nc.dram_tensor("iotas", [P, sw], FP32, kind="ExternalInput").ap()
    acc0 = nc.dram_tensor("acc0", [TPC * 128, D], FP32, kind="ExternalInput").ap()
    dinv2t = nc.dram_tensor("dinv2t", [P, TPC], FP32, kind="ExternalInput").ap()
    ckdt = nc.dram_tensor("ckdt", [P, K_HOPS * TPC], FP32, kind="ExternalInput").ap()
    wt = nc.dram_tensor("wt", [D, D], FP32, kind="ExternalInput").ap()
    bb = nc.dram_tensor("bb", [P, D], FP32, kind="ExternalInput").ap()
    y = nc.dram_tensor("y", [TPC * 128, D], FP32, kind="ExternalOutput").ap()

    with tile.TileContext(nc) as tc:
        with tc.tile_pool(name="const", bufs=1) as cpool, \
             tc.tile_pool(name="gp", bufs=6) as gpool, \
             tc.tile_pool(name="sp", bufs=4) as spool, \
             tc.tile_pool(name="fin", bufs=4) as fpool, \
             tc.tile_pool(name="ps", bufs=6, space="PSUM") as pspool, \
             tc.tile_pool(name="ps2", bufs=1, space="PSUM") as pspool2, \
             tc.tile_pool(name="dram", bufs=1, space="DRAM") as dram:

            # ---- persistent SBUF state ----
            acc = cpool.tile([P, TPC * 128], FP32)       # out accumulator
            nc.sync.dma_start(
                out=acc[:].rearrange("p (t f) -> p t f", f=128),
                in_=acc0.rearrange("(t p) f -> p t f", p=128))
            dinv2_sb = cpool.tile([P, TPC], FP32)
            nc.sync.dma_start(out=dinv2_sb[:], in_=dinv2t[:])
            ckd_sb = cpool.tile([P, K_HOPS * TPC], FP32)
            nc.sync.dma_start(out=ckd_sb[:], in_=ckdt[:])
            wt_sb = cpool.tile([D, D], FP32)
            nc.sync.dma_start(out=wt_sb[:], in_=wt[:])
            bb_sb = cpool.tile([P, D], FP32)
            nc.sync.dma_start(out=bb_sb[:], in_=bb[:])
            ident = cpool.tile([P, P], FP32)
            make_identity(nc, ident[:])
            part_acc = cpool.tile([P, TPC * 128], FP32)
            # resident gather indices, dst-locations, iota strip
            idx_sb = cpool.tile([P, nchunk * 8], I16)
            nc.sync.dma_start(out=idx_sb[:], in_=idx_d[:])
            dloc_sb = cpool.tile([P, nchunk], FP32)
            nc.sync.dma_start(out=dloc_sb[:], in_=dloc_d[:])
            iota_sb = cpool.tile([P, sw], FP32)
            nc.sync.dma_start(out=iota_sb[:], in_=iota_d[:])

            # ---- internal DRAM: alternating gather tables + AG inputs ----
            tA_int = [dram.tile([TA_ROWS, D], BF16, name=f"tAi{i}", tag=f"tAi{i}", addr_space="Shared") for i in range(K_HOPS)]
            tB_int = [dram.tile([TB_ROWS, D], BF16, name=f"tBi{i}", tag=f"tBi{i}", addr_space="Shared") for i in range(K_HOPS)]
            gnA = dram.tile([HA, D], BF16, tag="gnA")
            gnB = dram.tile([HB, D], BF16, tag="gnB")

            for k in range(1, K_HOPS + 1):
                if k == 1:
                    rdA, rdB = tA0, tB0
                else:
                    rdA, rdB = tA_int[k - 1][:], tB_int[k - 1][:]
                for ph, rd in ((0, rdA), (1, rdB)):
                    for bi, tiles in enumerate(_BATCHES):
                        nt = len(tiles)
                        nch = nt * cps
                        c0 = ph * TPC * cps + tiles[0] * cps
                        S = spool.tile([P, nch * 128], BF16, tag="S")
                        nc.vector.tensor_tensor(
                            out=S[:].rearrange("p (c i) -> p c i", i=128),
                            in0=iota_sb[:, :nch * 128].rearrange(
                                "p (c i) -> p c i", i=128),
                            in1=dloc_sb[:, c0:c0 + nch].unsqueeze(2)
                                .broadcast_to([P, nch, 128]),
                            op=mybir.AluOpType.is_equal)
                        G = gpool.tile([P, nch, 128], BF16, tag="G")
                        nc.gpsimd.dma_gather(
                            out_ap=G[:], in_ap=rd,
                            idxs_ap=idx_sb[:, c0 * 8:(c0 + nch) * 8],
                            num_idxs=nch * 128, num_idxs_reg=nch * 128,
                            elem_size=128, single_packet=False,
                            queue_num=bi % 4)
                        for si, t in enumerate(tiles):
                            ps = pspool.tile([P, D], FP32, tag="ps")
                            tc0 = t * 128
                            if ph == 1:
                                # fold phase-A partials in via identity matmul
                                nc.tensor.matmul(
                                    ps[:], lhsT=ident[:],
                                    rhs=part_acc[:, tc0:tc0 + 128],
                                    start=True, stop=False)
                            for j in range(cps):
                                sc = (si * cps + j) * 128
                                nc.tensor.matmul(
                                    ps[:], lhsT=S[:, sc:sc + 128],
                                    rhs=G[:, si * cps + j, :],
                                    start=(ph == 0 and j == 0),
                                    stop=(j == cps - 1))
                            if ph == 0:
                                # stash phase-A partial sums
                                nc.scalar.copy(part_acc[:, tc0:tc0 + 128], ps[:])
                                continue
                            # phase B: psum now holds the full segment sum
                            t1 = ps
                            if k < K_HOPS:
                                gnx = fpool.tile([P, D], BF16, tag="gnx")
                                nc.scalar.activation(
                                    out=gnx[:], in_=t1[:],
                                    func=mybir.ActivationFunctionType.Copy,
                                    scale=dinv2_sb[:, t:t + 1])
                                if t < A_TILES:
                                    nc.scalar.dma_start(
                                        out=gnA[t * 128:(t + 1) * 128, :], in_=gnx[:])
                                elif t < TPC - 1:
                                    r0 = t * 128 - HA
                                    nc.scalar.dma_start(
                                        out=gnB[r0:r0 + 128, :], in_=gnx[:])
                                else:
                                    r0 = t * 128 - HA
                                    nc.scalar.dma_start(
                                        out=gnB[r0:r0 + 106, :], in_=gnx[:106, :])
                            t2 = fpool.tile([P, D], FP32, tag="t2")
                            nc.scalar.activation(
                                out=t2[:], in_=t1[:],
                                func=mybir.ActivationFunctionType.Copy,
                                scale=ckd_sb[:, (k - 1) * TPC + t:(k - 1) * TPC + t + 1])
                            nc.vector.tensor_add(
                                acc[:, tc0:tc0 + 128], acc[:, tc0:tc0 + 128], t2[:])
                            if k < K_HOPS and t == A_TILES - 1:
                                nc.gpsimd.collective_compute(
                                    "AllGather", mybir.AluOpType.bypass,
                                    replica_groups=[list(range(N_CORES))],
                                    ins=[gnA[:].opt()],
                                    outs=[tA_int[k][:].opt()])
                            if k < K_HOPS and t == TPC - 1:
                                nc.gpsimd.collective_compute(
                                    "AllGather", mybir.AluOpType.bypass,
                                    replica_groups=[list(range(N_CORES))],
                                    ins=[gnB[:].opt()],
                                    outs=[tB_int[k][:].opt()])

            # ---- final linear: y = acc @ W.T + b ----
            for t in range(TPC):
                tc0 = t * 128
                pst = pspool2.tile([P, P], FP32, tag="pst")
                nc.tensor.transpose(
                    out=pst[:], in_=acc[:, tc0:tc0 + 128], identity=ident[:])
                accT = fpool.tile([P, P], FP32, tag="accT")
                nc.vector.tensor_copy(accT[:], pst[:])
                yps = pspool2.tile([P, D], FP32, tag="yps")
                nc.tensor.matmul(yps[:], lhsT=accT[:], rhs=wt_sb[:],
                                 start=True, stop=True)
                ysb = fpool.tile([P, D], FP32, tag="ysb")
                nc.vector.tensor_add(ysb[:], yps[:], bb_sb[:])
                nc.sync.dma_start(out=y[tc0:tc0 + 128, :], in_=ysb[:])
    nc.compile()
    return nc


def _wrap_idx(flat):
    """[n] int16 -> [128, n//16] wrapped (i -> partition i%16, col i//16),
    replicated to the 8 groups of 16 partitions."""
    n = flat.shape[0]
    w = flat.reshape(n // 16, 16).T  # [16, n//16]
    return np.tile(w, (8, 1))


def _preprocess(x, edge_index, t, W, b):
    x = np.asarray(x, dtype=np.float32)
    ei = np.asarray(edge_index)
    t = np.float32(np.asarray(t))
    W = np.asarray(W, dtype=np.float32)
    b = np.asarray(b, dtype=np.float32)

    src = np.concatenate([ei[0], np.arange(N, dtype=ei.dtype)]).astype(np.int64)
    dst = np.concatenate([ei[1], np.arange(N, dtype=ei.dtype)]).astype(np.int64)
    deg = np.bincount(dst, minlength=N).astype(np.float32)
    dinv = np.where(deg > 0, 1.0 / np.sqrt(deg), 0.0).astype(np.float32)

    # heat-kernel coefficients, computed like the reference (f32 chain)
    coeffs = np.zeros(K_HOPS + 1, dtype=np.float32)
    c = np.exp(-t).astype(np.float32) if hasattr(np.exp(-t), "astype") else np.float32(np.exp(-t))
    coeffs[0] = c
    for k in range(1, K_HOPS + 1):
        c = np.float32(c * t / np.float32(k))
        coeffs[k] = c

    g0 = (dinv[:, None] * x).astype(BF)

    # gather-table row id for each global node
    region = np.arange(N) // RPC
    off = np.arange(N) % RPC
    in_a = off < HA
    trow = np.where(in_a, region * HA + off, region * HB + (off - HA)).astype(np.int64)

    # rank-major tables
    g0r = g0.reshape(N_CORES, RPC, D)
    tA0 = np.ascontiguousarray(g0r[:, :HA].reshape(TA_ROWS, D))
    tB0 = np.ascontiguousarray(g0r[:, HA:].reshape(TB_ROWS, D))

    # per-core edge slotting
    e_src = src
    e_dst = dst
    e_core = e_dst // RPC
    e_loc = e_dst % RPC
    e_tile = e_loc // 128
    e_dloc = e_loc % 128
    e_half = (e_src % RPC < HA).astype(np.int64)  # 1 = A
    e_trow = trow[e_src]

    # determine max chunks per slot across all cores
    key = ((e_core * TPC + e_tile) * 2 + (1 - e_half))
    slot_counts = np.bincount(key, minlength=N_CORES * TPC * 2)
    cps = max(CPS, int(np.ceil(slot_counts.max() / 128)))
    slot = cps * 128
    nchunk = TPC * 2 * cps

    order = np.argsort(key, kind="stable")
    key_s = key[order]
    trow_s = e_trow[order]
    dloc_s = e_dloc[order]
    starts = np.searchsorted(key_s, np.arange(N_CORES * TPC * 2))
    ends = np.searchsorted(key_s, np.arange(N_CORES * TPC * 2), side="right")

    in_maps = []
    for c_ in range(N_CORES):
        # slot-major edge arrays, padded
        idx_slots = np.zeros((TPC * 2, slot), dtype=np.int16)
        dloc_slots = np.full((TPC * 2, slot), -1, dtype=np.int32)
        for ti in range(TPC):
            for h in range(2):  # 0 = A-half, 1 = B-half
                kidx = (c_ * TPC + ti) * 2 + h
                s0, s1 = starts[kidx], ends[kidx]
                cnt = s1 - s0
                sl = ti * 2 + h
                # ascending src rows -> gather descriptors walk DRAM forward
                seg = np.argsort(trow_s[s0:s1], kind="stable")
                idx_slots[sl, :cnt] = trow_s[s0:s1][seg].astype(np.int16)
                dloc_slots[sl, :cnt] = dloc_s[s0:s1][seg]

        # batch-major streams. batch bi covers tiles 4bi..4bi+3:
        # [A slots of all tiles in batch] then [B slots]
        idx_cols = []
        s_blocks = []
        for h in range(2):
            for tiles in _BATCHES:
                for ti in tiles:
                    sl = ti * 2 + h
                    idx_cols.append(_wrap_idx(idx_slots[sl]))
                    s_blocks.append(dloc_slots[sl])
        idx_np = np.concatenate(idx_cols, axis=1)  # [128, nchunk*8]
        dloc_all = np.concatenate(s_blocks)        # [nchunk*128]
        dl = dloc_all.reshape(nchunk, 128)
        dloc_np = np.ascontiguousarray(dl.T).astype(np.float32)  # [128, nchunk]
        sw = TILE_BATCH * cps * 128
        iota_np = np.tile(np.arange(128, dtype=np.float32)[None, :],
                          (P, sw // 128))

        r0 = c_ * RPC
        acc0 = np.zeros((TPC * 128, D), dtype=np.float32)
        acc0[:RPC] = coeffs[0] * x[r0:r0 + RPC]
        dinv_loc = np.zeros(TPC * 128, dtype=np.float32)
        dinv_loc[:RPC] = dinv[r0:r0 + RPC]
        dinv2t = np.ascontiguousarray(
            (dinv_loc * dinv_loc).reshape(TPC, 128).T)  # [128, TPC]
        ckdt = np.zeros((P, K_HOPS * TPC), dtype=np.float32)
        for k in range(1, K_HOPS + 1):
            ckdt[:, (k - 1) * TPC:k * TPC] = \
                (coeffs[k] * dinv_loc).reshape(TPC, 128).T
        in_maps.append({
            "tA0": tA0, "tB0": tB0,
            "idx": idx_np, "dlocs": dloc_np, "iotas": iota_np,
            "acc0": acc0, "dinv2t": dinv2t, "ckdt": ckdt,
            "wt": np.ascontiguousarray(W.T),
            "bb": np.tile(b[None, :], (P, 1)).astype(np.float32),
        })
    return in_maps, cps


_CACHE = {}


def kernel(x, edge_index, t, W, b):
    in_maps, cps = _preprocess(x, edge_index, t, W, b)
    if cps not in _CACHE:
        _CACHE[cps] = _build_program(cps)
    nc = _CACHE[cps]
    res = run_bass_kernel_spmd(nc, in_maps, core_ids=list(range(N_CORES)))
    out = np.empty((N, D), dtype=np.float32)
    for c_ in range(N_CORES):
        out[c_ * RPC:(c_ + 1) * RPC] = res.results[c_]["y"][:RPC]
    return out



# revision 10
# speedup vs baseline: 1.0304x; 1.0304x over previous
"""GCN heat-kernel diffusion (10 hops) + Linear on 8 Trainium2 NeuronCores.

Algorithm (matches reference):
    A_hat = D^-1/2 (Adj + I) D^-1/2
    out = (e^-t * sum_k t^k/k! A_hat^k x) @ W.T + b

Device mapping:
  - Nodes sharded across 8 cores (6250 dst rows each, 49 tiles of <=128).
  - State g_k = dinv * h_k kept replicated in DRAM as two bf16 tables
    (node halves, so gather indices fit int16):
        tableA rows: rank-major [8 x 3072], tableB: [8 x 3178].
  - Per hop per core: dma_gather g[src] rows (4 SWDGE queues), segment-sum
    via one-hot matmuls on the TensorEngine (S matrices precomputed on host
    and streamed from DRAM), finalize h = dinv*(segsum), accumulate
    out += c_k*h, write g_next = dinv*h, AllGather the two table halves.
  - Final: out @ W.T + b via PE transpose + matmul.
"""
import sys

sys.path.insert(0, "/opt/trn_rl_repo")

import numpy as np
import ml_dtypes

import concourse.bass as bass
import concourse.bacc as bacc
import concourse.tile as tile
from concourse import mybir
from concourse.bass_utils import run_bass_kernel_spmd
from concourse.masks import make_identity

FP32 = mybir.dt.float32
BF16 = mybir.dt.bfloat16
I16 = mybir.dt.int16
BF = ml_dtypes.bfloat16

N_CORES = 8
N = 50000
D = 128
K_HOPS = 10
RPC = N // N_CORES            # 6250 rows per core
TPC = 49                      # dst tiles per core (48*128 + 106)
HA = 3072                     # rows of each core region in table A (24 tiles)
HB = RPC - HA                 # 3178 rows in table B
TA_ROWS = N_CORES * HA        # 24576
TB_ROWS = N_CORES * HB        # 25424
A_TILES = HA // 128           # 24 tiles fully in half A
CPS = 8                       # chunks per (tile, half) slot: 1024 edge slots
SLOT = CPS * 128
TILE_BATCH = 4                # tiles per gather batch
P = 128

_BATCHES = [list(range(b, min(b + TILE_BATCH, TPC)))
            for b in range(0, TPC, TILE_BATCH)]  # 13 batches: 12x4 + 1x1


def _build_program(cps):
    slot = cps * 128
    nc = bacc.Bacc("TRN2", target_bir_lowering=False, debug=False,
                   num_devices=N_CORES, num_swdge_queues=4)
    nchunk = TPC * 2 * cps
    sw = TILE_BATCH * cps * 128   # widest S strip (4096 cols)
    tA0 = nc.dram_tensor("tA0", [TA_ROWS, D], BF16, kind="ExternalInput").ap()
    tB0 = nc.dram_tensor("tB0", [TB_ROWS, D], BF16, kind="ExternalInput").ap()
    idx_d = nc.dram_tensor("idx", [P, nchunk * 8], I16, kind="ExternalInput").ap()
    dloc_d = nc.dram_tensor("dlocs", [P, nchunk], FP32, kind="ExternalInput").ap()
    iota_d = nc.dram_tensor("iotas", [P, sw], FP32, kind="ExternalInput").ap()
    acc0 = nc.dram_tensor("acc0", [TPC * 128, D], FP32, kind="ExternalInput").ap()
    dinv2t = nc.dram_tensor("dinv2t", [P, TPC], FP32, kind="ExternalInput").ap()
    ckdt = nc.dram_tensor("ckdt", [P, K_HOPS * TPC], FP32, kind="ExternalInput").ap()
    wt = nc.dram_tensor("wt", [D, D], FP32, kind="ExternalInput").ap()
    bb = nc.dram_tensor("bb", [P, D], FP32, kind="ExternalInput").ap()
    y = nc.dram_tensor("y", [TPC * 128, D], FP32, kind="ExternalOutput").ap()

    with tile.TileContext(nc) as tc:
        with tc.tile_pool(name="const", bufs=1) as cpool, \
             tc.tile_pool(name="gp", bufs=6) as gpool, \
             tc.tile_pool(name="sp", bufs=4) as spool, \
             tc.tile_pool(name="fin", bufs=4) as fpool, \
             tc.tile_pool(name="ps", bufs=6, space="PSUM") as pspool, \
             tc.tile_pool(name="ps2", bufs=1, space="PSUM") as pspool2, \
             tc.tile_pool(name="dram", bufs=1, space="DRAM") as dram:

            # ---- persistent SBUF state ----
            acc = cpool.tile([P, TPC * 128], FP32)       # out accumulator
            nc.sync.dma_start(
                out=acc[:].rearrange("p (t f) -> p t f", f=128),
                in_=acc0.rearrange("(t p) f -> p t f", p=128))
            dinv2_sb = cpool.tile([P, TPC], FP32)
            nc.sync.dma_start(out=dinv2_sb[:], in_=dinv2t[:])
            ckd_sb = cpool.tile([P, K_HOPS * TPC], FP32)
            nc.sync.dma_start(out=ckd_sb[:], in_=ckdt[:])
            wt_sb = cpool.tile([D, D], FP32)
            nc.sync.dma_start(out=wt_sb[:], in_=wt[:])
            bb_sb = cpool.tile([P, D], FP32)
            nc.sync.dma_start(out=bb_sb[:], in_=bb[:])
            ident = cpool.tile([P, P], FP32)
            make_identity(nc, ident[:])
            part_acc = cpool.tile([P, TPC * 128], FP32)
            # resident gather indices, dst-locations, iota strip
            idx_sb = cpool.tile([P, nchunk * 8], I16)
            nc.sync.dma_start(out=idx_sb[:], in_=idx_d[:])
            dloc_sb = cpool.tile([P, nchunk], FP32)
            nc.sync.dma_start(out=dloc_sb[:], in_=dloc_d[:])
            iota_sb = cpool.tile([P, sw], FP32)
            nc.sync.dma_start(out=iota_sb[:], in_=iota_d[:])

            # ---- internal DRAM: alternating gather tables + AG inputs ----
            tA_int = [dram.tile([TA_ROWS, D], BF16, name=f"tAi{i}", tag=f"tAi{i}", addr_space="Shared") for i in range(K_HOPS)]
            tB_int = [dram.tile([TB_ROWS, D], BF16, name=f"tBi{i}", tag=f"tBi{i}", addr_space="Shared") for i in range(K_HOPS)]
            gnA = dram.tile([HA, D], BF16, tag="gnA")
            gnB = dram.tile([HB, D], BF16, tag="gnB")

            call_no = 0
            for k in range(1, K_HOPS + 1):
                if k == 1:
                    rdA, rdB = tA0, tB0
                else:
                    rdA, rdB = tA_int[k - 1][:], tB_int[k - 1][:]
                for ph, rd in ((0, rdA), (1, rdB)):
                    for bi, tiles in enumerate(_BATCHES):
                        nt = len(tiles)
                        nch = nt * cps
                        c0 = ph * TPC * cps + tiles[0] * cps
                        S = spool.tile([P, nch * 128], BF16, tag="S")
                        nc.vector.tensor_tensor(
                            out=S[:].rearrange("p (c i) -> p c i", i=128),
                            in0=iota_sb[:, :nch * 128].rearrange(
                                "p (c i) -> p c i", i=128),
                            in1=dloc_sb[:, c0:c0 + nch].unsqueeze(2)
                                .broadcast_to([P, nch, 128]),
                            op=mybir.AluOpType.is_equal)
                        G = gpool.tile([P, nch, 128], BF16, tag="G")
                        nc.gpsimd.dma_gather(
                            out_ap=G[:], in_ap=rd,
                            idxs_ap=idx_sb[:, c0 * 8:(c0 + nch) * 8],
                            num_idxs=nch * 128, num_idxs_reg=nch * 128,
                            elem_size=128, single_packet=False,
                            queue_num=call_no % 4)
                        call_no += 1
                        for si, t in enumerate(tiles):
                            ps = pspool.tile([P, D], FP32, tag="ps")
                            tc0 = t * 128
                            if ph == 1:
                                # fold phase-A partials in via identity matmul
                                nc.tensor.matmul(
                                    ps[:], lhsT=ident[:],
                                    rhs=part_acc[:, tc0:tc0 + 128],
                                    start=True, stop=False)
                            for j in range(cps):
                                sc = (si * cps + j) * 128
                                nc.tensor.matmul(
                                    ps[:], lhsT=S[:, sc:sc + 128],
                                    rhs=G[:, si * cps + j, :],
                                    start=(ph == 0 and j == 0),
                                    stop=(j == cps - 1))
                            if ph == 0:
                                # stash phase-A partial sums
                                nc.scalar.copy(part_acc[:, tc0:tc0 + 128], ps[:])
                                continue
                            # phase B: psum now holds the full segment sum
                            t1 = ps
                            if k < K_HOPS:
                                gnx = fpool.tile([P, D], BF16, tag="gnx")
                                nc.scalar.activation(
                                    out=gnx[:], in_=t1[:],
                                    func=mybir.ActivationFunctionType.Copy,
                                    scale=dinv2_sb[:, t:t + 1])
                                if t < A_TILES:
                                    nc.scalar.dma_start(
                                        out=gnA[t * 128:(t + 1) * 128, :], in_=gnx[:])
                                elif t < TPC - 1:
                                    r0 = t * 128 - HA
                                    nc.scalar.dma_start(
                                        out=gnB[r0:r0 + 128, :], in_=gnx[:])
                                else:
                                    r0 = t * 128 - HA
                                    nc.scalar.dma_start(
                                        out=gnB[r0:r0 + 106, :], in_=gnx[:106, :])
                            t2 = fpool.tile([P, D], FP32, tag="t2")
                            nc.scalar.activation(
                                out=t2[:], in_=t1[:],
                                func=mybir.ActivationFunctionType.Copy,
                                scale=ckd_sb[:, (k - 1) * TPC + t:(k - 1) * TPC + t + 1])
                            nc.vector.tensor_add(
                                acc[:, tc0:tc0 + 128], acc[:, tc0:tc0 + 128], t2[:])
                            if k < K_HOPS and t == A_TILES - 1:
                                nc.gpsimd.collective_compute(
                                    "AllGather", mybir.AluOpType.bypass,
                                    replica_groups=[list(range(N_CORES))],
                                    ins=[gnA[:].opt()],
                                    outs=[tA_int[k][:].opt()])
                            if k < K_HOPS and t == TPC - 1:
                                nc.gpsimd.collective_compute(
                                    "AllGather", mybir.AluOpType.bypass,
                                    replica_groups=[list(range(N_CORES))],
                                    ins=[gnB[:].opt()],
                                    outs=[tB_int[k][:].opt()])

            # ---- final linear: y = acc @ W.T + b ----
            for t in range(TPC):
                tc0 = t * 128
                pst = pspool2.tile([P, P], FP32, tag="pst")
                nc.tensor.transpose(
                    out=pst[:], in_=acc[:, tc0:tc0 + 128], identity=ident[:])
                accT = fpool.tile([P, P], FP32, tag="accT")
                nc.vector.tensor_copy(accT[:], pst[:])
                yps = pspool2.tile([P, D], FP32, tag="yps")
                nc.tensor.matmul(yps[:], lhsT=accT[:], rhs=wt_sb[:],
                                 start=True, stop=True)
                ysb = fpool.tile([P, D], FP32, tag="ysb")
                nc.vector.tensor_add(ysb[:], yps[:], bb_sb[:])
                nc.sync.dma_start(out=y[tc0:tc0 + 128, :], in_=ysb[:])
    nc.compile()
    return nc


def _wrap_idx(flat):
    """[n] int16 -> [128, n//16] wrapped (i -> partition i%16, col i//16),
    replicated to the 8 groups of 16 partitions."""
    n = flat.shape[0]
    w = flat.reshape(n // 16, 16).T  # [16, n//16]
    return np.tile(w, (8, 1))


def _preprocess(x, edge_index, t, W, b):
    x = np.asarray(x, dtype=np.float32)
    ei = np.asarray(edge_index)
    t = np.float32(np.asarray(t))
    W = np.asarray(W, dtype=np.float32)
    b = np.asarray(b, dtype=np.float32)

    src = np.concatenate([ei[0], np.arange(N, dtype=ei.dtype)]).astype(np.int64)
    dst = np.concatenate([ei[1], np.arange(N, dtype=ei.dtype)]).astype(np.int64)
    deg = np.bincount(dst, minlength=N).astype(np.float32)
    dinv = np.where(deg > 0, 1.0 / np.sqrt(deg), 0.0).astype(np.float32)

    # heat-kernel coefficients, computed like the reference (f32 chain)
    coeffs = np.zeros(K_HOPS + 1, dtype=np.float32)
    c = np.exp(-t).astype(np.float32) if hasattr(np.exp(-t), "astype") else np.float32(np.exp(-t))
    coeffs[0] = c
    for k in range(1, K_HOPS + 1):
        c = np.float32(c * t / np.float32(k))
        coeffs[k] = c

    g0 = (dinv[:, None] * x).astype(BF)

    # gather-table row id for each global node
    region = np.arange(N) // RPC
    off = np.arange(N) % RPC
    in_a = off < HA
    trow = np.where(in_a, region * HA + off, region * HB + (off - HA)).astype(np.int64)

    # rank-major tables
    g0r = g0.reshape(N_CORES, RPC, D)
    tA0 = np.ascontiguousarray(g0r[:, :HA].reshape(TA_ROWS, D))
    tB0 = np.ascontiguousarray(g0r[:, HA:].reshape(TB_ROWS, D))

    # per-core edge slotting
    e_src = src
    e_dst = dst
    e_core = e_dst // RPC
    e_loc = e_dst % RPC
    e_tile = e_loc // 128
    e_dloc = e_loc % 128
    e_half = (e_src % RPC < HA).astype(np.int64)  # 1 = A
    e_trow = trow[e_src]

    # determine max chunks per slot across all cores
    key = ((e_core * TPC + e_tile) * 2 + (1 - e_half))
    slot_counts = np.bincount(key, minlength=N_CORES * TPC * 2)
    cps = max(CPS, int(np.ceil(slot_counts.max() / 128)))
    slot = cps * 128
    nchunk = TPC * 2 * cps

    order = np.argsort(key, kind="stable")
    key_s = key[order]
    trow_s = e_trow[order]
    dloc_s = e_dloc[order]
    starts = np.searchsorted(key_s, np.arange(N_CORES * TPC * 2))
    ends = np.searchsorted(key_s, np.arange(N_CORES * TPC * 2), side="right")

    in_maps = []
    for c_ in range(N_CORES):
        # slot-major edge arrays, padded
        idx_slots = np.zeros((TPC * 2, slot), dtype=np.int16)
        dloc_slots = np.full((TPC * 2, slot), -1, dtype=np.int32)
        for ti in range(TPC):
            for h in range(2):  # 0 = A-half, 1 = B-half
                kidx = (c_ * TPC + ti) * 2 + h
                s0, s1 = starts[kidx], ends[kidx]
                cnt = s1 - s0
                sl = ti * 2 + h
                # ascending src rows -> gather descriptors walk DRAM forward
                seg = np.argsort(trow_s[s0:s1], kind="stable")
                idx_slots[sl, :cnt] = trow_s[s0:s1][seg].astype(np.int16)
                dloc_slots[sl, :cnt] = dloc_s[s0:s1][seg]

        # batch-major streams. batch bi covers tiles 4bi..4bi+3:
        # [A slots of all tiles in batch] then [B slots]
        idx_cols = []
        s_blocks = []
        for h in range(2):
            for tiles in _BATCHES:
                for ti in tiles:
                    sl = ti * 2 + h
                    idx_cols.append(_wrap_idx(idx_slots[sl]))
                    s_blocks.append(dloc_slots[sl])
        idx_np = np.concatenate(idx_cols, axis=1)  # [128, nchunk*8]
        dloc_all = np.concatenate(s_blocks)        # [nchunk*128]
        dl = dloc_all.reshape(nchunk, 128)
        dloc_np = np.ascontiguousarray(dl.T).astype(np.float32)  # [128, nchunk]
        sw = TILE_BATCH * cps * 128
        iota_np = np.tile(np.arange(128, dtype=np.float32)[None, :],
                          (P, sw // 128))

        r0 = c_ * RPC
        acc0 = np.zeros((TPC * 128, D), dtype=np.float32)
        acc0[:RPC] = coeffs[0] * x[r0:r0 + RPC]
        dinv_loc = np.zeros(TPC * 128, dtype=np.float32)
        dinv_loc[:RPC] = dinv[r0:r0 + RPC]
        dinv2t = np.ascontiguousarray(
            (dinv_loc * dinv_loc).reshape(TPC, 128).T)  # [128, TPC]
        ckdt = np.zeros((P, K_HOPS * TPC), dtype=np.float32)
        for k in range(1, K_HOPS + 1):
            ckdt[:, (k - 1) * TPC:k * TPC] = \
                (coeffs[k] * dinv_loc).reshape(TPC, 128).T
        in_maps.append({
            "tA0": tA0, "tB0": tB0,
            "idx": idx_np, "dlocs": dloc_np, "iotas": iota_np,
            "acc0": acc0, "dinv2t": dinv2t, "ckdt": ckdt,
            "wt": np.ascontiguousarray(W.T),
            "bb": np.tile(b[None, :], (P, 1)).astype(np.float32),
        })
    return in_maps, cps


_CACHE = {}


def kernel(x, edge_index, t, W, b):
    in_maps, cps = _preprocess(x, edge_index, t, W, b)
    if cps not in _CACHE:
        _CACHE[cps] = _build_program(cps)
    nc = _CACHE[cps]
    res = run_bass_kernel_spmd(nc, in_maps, core_ids=list(range(N_CORES)))
    out = np.empty((N, D), dtype=np.float32)
    for c_ in range(N_CORES):
        out[c_ * RPC:(c_ + 1) * RPC] = res.results[c_]["y"][:RPC]
    return out



# revision 12
# speedup vs baseline: 1.3770x; 1.3363x over previous
"""GCN heat-kernel diffusion (10 hops) + Linear on 8 Trainium2 NeuronCores.

v4 = v2 (on-chip one-hot S, resident idx/dloc, balanced queues) + densely
packed chunks: per (tile, half) the chunk count is ceil(max-over-cores
count / 128) instead of a fixed 8, with per-core shortfall as trailing -1
idx (trimmed by the gather ucode). Self-loops gathered as in v2.
"""
import sys

sys.path.insert(0, "/opt/trn_rl_repo")

import numpy as np
import ml_dtypes

import concourse.bass as bass
import concourse.bacc as bacc
import concourse.tile as tile
from concourse import mybir
from concourse.bass_utils import run_bass_kernel_spmd
from concourse.masks import make_identity

FP32 = mybir.dt.float32
BF16 = mybir.dt.bfloat16
I16 = mybir.dt.int16
BF = ml_dtypes.bfloat16

N_CORES = 8
N = 50000
D = 128
K_HOPS = 10
RPC = N // N_CORES            # 6250 rows per core
TPC = 49                      # dst tiles per core (48*128 + 106)
HA = 3072                     # rows of each core region in table A (24 tiles)
HB = RPC - HA                 # 3178 rows in table B
TA_ROWS = N_CORES * HA        # 24576
TB_ROWS = N_CORES * HB        # 25424
A_TILES = HA // 128           # 24 tiles fully in half A
TILE_BATCH = 4                # tiles per gather batch
P = 128

_BATCHES = [list(range(b, min(b + TILE_BATCH, TPC)))
            for b in range(0, TPC, TILE_BATCH)]  # 13 batches: 12x4 + 1x1


def _build_program(U):
    """U: [TPC][2] chunk counts per (tile, half: 0=A, 1=B)."""
    nc = bacc.Bacc("TRN2", target_bir_lowering=False, debug=False,
                   num_devices=N_CORES, num_swdge_queues=4)
    nchunk = int(sum(U[t][h] for t in range(TPC) for h in range(2)))
    maxw = max(sum(U[t][h] for t in tiles) for tiles in _BATCHES
               for h in range(2)) * 128
    tA0 = nc.dram_tensor("tA0", [TA_ROWS, D], BF16, kind="ExternalInput").ap()
    tB0 = nc.dram_tensor("tB0", [TB_ROWS, D], BF16, kind="ExternalInput").ap()
    idx_d = nc.dram_tensor("idx", [P, nchunk * 8], I16, kind="ExternalInput").ap()
    dloc_d = nc.dram_tensor("dlocs", [P, nchunk], FP32, kind="ExternalInput").ap()
    iota_d = nc.dram_tensor("iotas", [P, maxw], FP32, kind="ExternalInput").ap()
    acc0 = nc.dram_tensor("acc0", [TPC * 128, D], FP32, kind="ExternalInput").ap()
    dinv2t = nc.dram_tensor("dinv2t", [P, TPC], FP32, kind="ExternalInput").ap()
    ckdt = nc.dram_tensor("ckdt", [P, K_HOPS * TPC], FP32, kind="ExternalInput").ap()
    wt = nc.dram_tensor("wt", [D, D], FP32, kind="ExternalInput").ap()
    bb = nc.dram_tensor("bb", [P, D], FP32, kind="ExternalInput").ap()
    y = nc.dram_tensor("y", [TPC * 128, D], FP32, kind="ExternalOutput").ap()

    # cumulative chunk offsets in (ph, batch, tile) stream order
    coff = {}
    c = 0
    for ph in range(2):
        for tiles in _BATCHES:
            for t in tiles:
                coff[(ph, t)] = c
                c += U[t][ph]
    assert c == nchunk

    with tile.TileContext(nc) as tc:
        with tc.tile_pool(name="const", bufs=1) as cpool, \
             tc.tile_pool(name="gp", bufs=6) as gpool, \
             tc.tile_pool(name="sp", bufs=4) as spool, \
             tc.tile_pool(name="fin", bufs=4) as fpool, \
             tc.tile_pool(name="ps", bufs=6, space="PSUM") as pspool, \
             tc.tile_pool(name="ps2", bufs=1, space="PSUM") as pspool2, \
             tc.tile_pool(name="dram", bufs=1, space="DRAM") as dram:

            # ---- persistent SBUF state ----
            acc = cpool.tile([P, TPC * 128], FP32)       # out accumulator
            nc.sync.dma_start(
                out=acc[:].rearrange("p (t f) -> p t f", f=128),
                in_=acc0.rearrange("(t p) f -> p t f", p=128))
            dinv2_sb = cpool.tile([P, TPC], FP32)
            nc.sync.dma_start(out=dinv2_sb[:], in_=dinv2t[:])
            ckd_sb = cpool.tile([P, K_HOPS * TPC], FP32)
            nc.sync.dma_start(out=ckd_sb[:], in_=ckdt[:])
            wt_sb = cpool.tile([D, D], FP32)
            nc.sync.dma_start(out=wt_sb[:], in_=wt[:])
            bb_sb = cpool.tile([P, D], FP32)
            nc.sync.dma_start(out=bb_sb[:], in_=bb[:])
            ident = cpool.tile([P, P], FP32)
            make_identity(nc, ident[:])
            part_acc = cpool.tile([P, TPC * 128], FP32)
            # resident gather indices, dst-locations, iota strip
            idx_sb = cpool.tile([P, nchunk * 8], I16)
            nc.sync.dma_start(out=idx_sb[:], in_=idx_d[:])
            dloc_sb = cpool.tile([P, nchunk], FP32)
            nc.sync.dma_start(out=dloc_sb[:], in_=dloc_d[:])
            iota_sb = cpool.tile([P, maxw], FP32)
            nc.sync.dma_start(out=iota_sb[:], in_=iota_d[:])

            # ---- internal DRAM: alternating gather tables + AG inputs ----
            tA_int = [dram.tile([TA_ROWS, D], BF16, name=f"tAi{i}", tag=f"tAi{i}", addr_space="Shared") for i in range(K_HOPS)]
            tB_int = [dram.tile([TB_ROWS, D], BF16, name=f"tBi{i}", tag=f"tBi{i}", addr_space="Shared") for i in range(K_HOPS)]
            gnA = dram.tile([HA, D], BF16, tag="gnA")
            gnB = dram.tile([HB, D], BF16, tag="gnB")

            call_no = 0
            for k in range(1, K_HOPS + 1):
                if k == 1:
                    rdA, rdB = tA0, tB0
                else:
                    rdA, rdB = tA_int[k - 1][:], tB_int[k - 1][:]
                for ph, rd in ((0, rdA), (1, rdB)):
                    for bi, tiles in enumerate(_BATCHES):
                        nch = sum(U[t][ph] for t in tiles)
                        c0 = coff[(ph, tiles[0])]
                        S = spool.tile([P, nch * 128], BF16, tag="S")
                        nc.vector.tensor_tensor(
                            out=S[:].rearrange("p (c i) -> p c i", i=128),
                            in0=iota_sb[:, :nch * 128].rearrange(
                                "p (c i) -> p c i", i=128),
                            in1=dloc_sb[:, c0:c0 + nch].unsqueeze(2)
                                .broadcast_to([P, nch, 128]),
                            op=mybir.AluOpType.is_equal)
                        G = gpool.tile([P, nch, 128], BF16, tag="G")
                        nc.gpsimd.dma_gather(
                            out_ap=G[:], in_ap=rd,
                            idxs_ap=idx_sb[:, c0 * 8:(c0 + nch) * 8],
                            num_idxs=nch * 128, num_idxs_reg=nch * 128,
                            elem_size=128, single_packet=False,
                            queue_num=call_no % 4)
                        call_no += 1
                        for t in tiles:
                            u = U[t][ph]
                            base = coff[(ph, t)] - c0
                            ps = pspool.tile([P, D], FP32, tag="ps")
                            tc0 = t * 128
                            if ph == 1:
                                # fold phase-A partials in via identity matmul
                                nc.tensor.matmul(
                                    ps[:], lhsT=ident[:],
                                    rhs=part_acc[:, tc0:tc0 + 128],
                                    start=True, stop=False)
                            for j in range(u):
                                sc = (base + j) * 128
                                nc.tensor.matmul(
                                    ps[:], lhsT=S[:, sc:sc + 128],
                                    rhs=G[:, base + j, :],
                                    start=(ph == 0 and j == 0),
                                    stop=(j == u - 1))
                            if ph == 0:
                                # stash phase-A partial sums
                                nc.scalar.copy(part_acc[:, tc0:tc0 + 128], ps[:])
                                continue
                            # phase B: psum now holds the full segment sum
                            t1 = ps
                            if k < K_HOPS:
                                gnx = fpool.tile([P, D], BF16, tag="gnx")
                                nc.scalar.activation(
                                    out=gnx[:], in_=t1[:],
                                    func=mybir.ActivationFunctionType.Copy,
                                    scale=dinv2_sb[:, t:t + 1])
                                if t < A_TILES:
                                    nc.scalar.dma_start(
                                        out=gnA[t * 128:(t + 1) * 128, :], in_=gnx[:])
                                elif t < TPC - 1:
                                    r0 = t * 128 - HA
                                    nc.scalar.dma_start(
                                        out=gnB[r0:r0 + 128, :], in_=gnx[:])
                                else:
                                    r0 = t * 128 - HA
                                    nc.scalar.dma_start(
                                        out=gnB[r0:r0 + 106, :], in_=gnx[:106, :])
                            t2 = fpool.tile([P, D], FP32, tag="t2")
                            nc.scalar.activation(
                                out=t2[:], in_=t1[:],
                                func=mybir.ActivationFunctionType.Copy,
                                scale=ckd_sb[:, (k - 1) * TPC + t:(k - 1) * TPC + t + 1])
                            nc.vector.tensor_add(
                                acc[:, tc0:tc0 + 128], acc[:, tc0:tc0 + 128], t2[:])
                            if k < K_HOPS and t == A_TILES - 1:
                                nc.gpsimd.collective_compute(
                                    "AllGather", mybir.AluOpType.bypass,
                                    replica_groups=[list(range(N_CORES))],
                                    ins=[gnA[:].opt()],
                                    outs=[tA_int[k][:].opt()])
                            if k < K_HOPS and t == TPC - 1:
                                nc.gpsimd.collective_compute(
                                    "AllGather", mybir.AluOpType.bypass,
                                    replica_groups=[list(range(N_CORES))],
                                    ins=[gnB[:].opt()],
                                    outs=[tB_int[k][:].opt()])

            # ---- final linear: y = acc @ W.T + b ----
            for t in range(TPC):
                tc0 = t * 128
                pst = pspool2.tile([P, P], FP32, tag="pst")
                nc.tensor.transpose(
                    out=pst[:], in_=acc[:, tc0:tc0 + 128], identity=ident[:])
                accT = fpool.tile([P, P], FP32, tag="accT")
                nc.vector.tensor_copy(accT[:], pst[:])
                yps = pspool2.tile([P, D], FP32, tag="yps")
                nc.tensor.matmul(yps[:], lhsT=accT[:], rhs=wt_sb[:],
                                 start=True, stop=True)
                ysb = fpool.tile([P, D], FP32, tag="ysb")
                nc.vector.tensor_add(ysb[:], yps[:], bb_sb[:])
                nc.sync.dma_start(out=y[tc0:tc0 + 128, :], in_=ysb[:])
    nc.compile()
    return nc


def _wrap_idx(flat):
    """[n] int16 -> [128, n//16] wrapped (i -> partition i%16, col i//16),
    replicated to the 8 groups of 16 partitions."""
    n = flat.shape[0]
    w = flat.reshape(n // 16, 16).T  # [16, n//16]
    return np.tile(w, (8, 1))


def _preprocess(x, edge_index, t, W, b):
    x = np.asarray(x, dtype=np.float32)
    ei = np.asarray(edge_index)
    t = np.float32(np.asarray(t))
    W = np.asarray(W, dtype=np.float32)
    b = np.asarray(b, dtype=np.float32)

    src = np.concatenate([ei[0], np.arange(N, dtype=ei.dtype)]).astype(np.int64)
    dst = np.concatenate([ei[1], np.arange(N, dtype=ei.dtype)]).astype(np.int64)
    deg = np.bincount(dst, minlength=N).astype(np.float32)
    dinv = np.where(deg > 0, 1.0 / np.sqrt(deg), 0.0).astype(np.float32)

    # heat-kernel coefficients, computed like the reference (f32 chain)
    coeffs = np.zeros(K_HOPS + 1, dtype=np.float32)
    c = np.float32(np.exp(-t))
    coeffs[0] = c
    for k in range(1, K_HOPS + 1):
        c = np.float32(c * t / np.float32(k))
        coeffs[k] = c

    g0 = (dinv[:, None] * x).astype(BF)

    # gather-table row id for each global node
    region = np.arange(N) // RPC
    off = np.arange(N) % RPC
    in_a = off < HA
    trow = np.where(in_a, region * HA + off, region * HB + (off - HA)).astype(np.int64)

    # rank-major tables
    g0r = g0.reshape(N_CORES, RPC, D)
    tA0 = np.ascontiguousarray(g0r[:, :HA].reshape(TA_ROWS, D))
    tB0 = np.ascontiguousarray(g0r[:, HA:].reshape(TB_ROWS, D))

    # per-core edge slotting
    e_core = dst // RPC
    e_loc = dst % RPC
    e_tile = e_loc // 128
    e_dloc = e_loc % 128
    e_half = (src % RPC < HA).astype(np.int64)  # 1 = A
    e_trow = trow[src]

    # per (core, tile, half) counts -> shared chunk counts U
    key = ((e_core * TPC + e_tile) * 2 + (1 - e_half))
    slot_counts = np.bincount(key, minlength=N_CORES * TPC * 2).reshape(
        N_CORES, TPC, 2)
    maxcnt = slot_counts.max(axis=0)             # [TPC, 2]
    U = np.maximum(1, np.ceil(maxcnt / 128).astype(np.int64))  # [TPC][2]

    order = np.argsort(key, kind="stable")
    key_s = key[order]
    trow_s = e_trow[order]
    dloc_s = e_dloc[order]
    starts = np.searchsorted(key_s, np.arange(N_CORES * TPC * 2))
    ends = np.searchsorted(key_s, np.arange(N_CORES * TPC * 2), side="right")

    nchunk = int(U.sum())
    maxw = max(int(sum(U[t][h] for t in tiles)) for tiles in _BATCHES
               for h in range(2)) * 128

    in_maps = []
    for c_ in range(N_CORES):
        idx_cols = []
        dloc_cols = []
        for ph in range(2):
            for tiles in _BATCHES:
                for ti in tiles:
                    u = int(U[ti][ph])
                    cap = u * 128
                    kidx = (c_ * TPC + ti) * 2 + ph
                    s0, s1 = starts[kidx], ends[kidx]
                    cnt = s1 - s0
                    # ascending src rows -> gather walks DRAM forward.
                    # Padding must point at a valid row (0): negative idx
                    # handling in the gather runtime proved unreliable.
                    seg = np.argsort(trow_s[s0:s1], kind="stable")
                    tr = np.zeros(cap, dtype=np.int16)
                    dl = np.full(cap, -1, dtype=np.int32)
                    tr[:cnt] = trow_s[s0:s1][seg].astype(np.int16)
                    dl[:cnt] = dloc_s[s0:s1][seg]
                    idx_cols.append(_wrap_idx(tr))
                    dloc_cols.append(dl.reshape(u, 128).T)  # [128, u]
        idx_np = np.concatenate(idx_cols, axis=1)          # [128, nchunk*8]
        dloc_np = np.ascontiguousarray(
            np.concatenate(dloc_cols, axis=1)).astype(np.float32)
        iota_np = np.tile(np.arange(128, dtype=np.float32)[None, :],
                          (P, maxw // 128))

        r0 = c_ * RPC
        acc0 = np.zeros((TPC * 128, D), dtype=np.float32)
        acc0[:RPC] = coeffs[0] * x[r0:r0 + RPC]
        dinv_loc = np.zeros(TPC * 128, dtype=np.float32)
        dinv_loc[:RPC] = dinv[r0:r0 + RPC]
        dinv2t = np.ascontiguousarray(
            (dinv_loc * dinv_loc).reshape(TPC, 128).T)  # [128, TPC]
        ckdt = np.zeros((P, K_HOPS * TPC), dtype=np.float32)
        for k in range(1, K_HOPS + 1):
            ckdt[:, (k - 1) * TPC:k * TPC] = \
                (coeffs[k] * dinv_loc).reshape(TPC, 128).T
        in_maps.append({
            "tA0": tA0, "tB0": tB0,
            "idx": idx_np, "dlocs": dloc_np, "iotas": iota_np,
            "acc0": acc0, "dinv2t": dinv2t, "ckdt": ckdt,
            "wt": np.ascontiguousarray(W.T),
            "bb": np.tile(b[None, :], (P, 1)).astype(np.float32),
        })
    return in_maps, tuple(tuple(int(v) for v in row) for row in U)


_CACHE = {}


def kernel(x, edge_index, t, W, b):
    in_maps, U = _preprocess(x, edge_index, t, W, b)
    if U not in _CACHE:
        _CACHE[U] = _build_program(U)
    nc = _CACHE[U]
    res = run_bass_kernel_spmd(nc, in_maps, core_ids=list(range(N_CORES)))
    out = np.empty((N, D), dtype=np.float32)
    for c_ in range(N_CORES):
        out[c_ * RPC:(c_ + 1) * RPC] = res.results[c_]["y"][:RPC]
    return out
